# revision 32
# baseline (speedup 1.0000x reference)
"""Trainium2 Bass kernel for nn_CVAE decoder LSTM.

Data-parallel over 8 NeuronCores: batch 8192 -> 1024 per core.

Math (per core, B=1024, T=2048, H=128). Reference step:
    gates = x_t @ Wx.T + static_proj + h @ W_hh.T
    i,f,g,o = split(gates); c' = sig(f)*c + sig(i)*tanh(g)
    h' = sig(o)*tanh(c'); x' = h' @ W_out.T + b_out; ys[t] = x'
Folds applied (all exact):
  1. x_t = h_t @ W_out.T + b_out for t>=1:
       W_comb = W_hh.T + W_out.T @ Wx.T        [128, 512]
       P0     = static_proj + b_out @ Wx.T     [B, 512]
       gates_t = h_t @ W_comb + P0   (t>=1);  gates_0 = Pf (h_0 = 0)
  2. tanh(x) = 2*sigmoid(2x) - 1: double the g-gate columns of W_comb/P0 so
     ONE sigmoid instruction covers all 4 gates; the affine 2s-1 rides the
     scalar_tensor_tensor multiplies on DVE.
  3. h = sig(o)*tanh(c) = 2*(sig(2c)-0.5)*sig(o) =: 2*ht. The x2 is folded
     into W_comb and W_out (both doubled), so the kernel state is
     ht = (sig(2c)-0.5)*sig(o), one STT op.
Per group (BG=256) per step:
  PE:   4 gate matmuls (W_eff) + 4 identity matmuls (P0 accumulate)
  ACT:  s = sigmoid(gates) over [128,4,BG] (ONE instr); sc = sigmoid(2c')
  Pool: t2 = s[f]*c
  DVE:  v = (s[g]-0.5)*s[i] ; c' = 2v + t2 ; ht = (sc-0.5)*s[o]
  PE:   x-projection into a per-group PSUM window tile [128,2,U,2]
        (bias pre-filled via a K=1 ones-matmul), emitted in XB=8-step
        BURSTS after the ht ring-buffer slots are all written -- a
        per-step out-proj would park an LDWEIGHTS waiting on ht at the
        PE queue head and stall the other groups' gate matmuls behind
        it (measured +2.5 ms). Window copied PSUM->SBUF, DMA'd per U.
Batch is split into NG=4 independent groups of BG=256 whose serial cell
chains stagger across engines. Chain: PE -> ACT -> {Pool,DVE} -> DVE ->
ACT -> DVE -> PE. ACT is the throughput bound (sim: 99.6% busy); NG=2
halves ACT instr overhead but the longer per-group chain latency loses
more (measured), and merging sigmoid_c across groups couples chains
(also measured worse). Failed variants (measured slower): c-add on Pool
(17.3ms), XB=4 (15.9ms), no-P0-ident (17.9ms), sc pair-merge (sim 15.1).
"""

import os
import numpy as np

import concourse.bass as bass
import concourse.bacc as bacc
import concourse.tile as tile
from concourse import mybir
from concourse.bass_utils import run_bass_kernel_spmd

f32 = mybir.dt.float32
f32r = mybir.dt.float32r
AF = mybir.ActivationFunctionType
ALU = mybir.AluOpType

HIDDEN = 128
INPUT_SIZE = 2
T = 2048
B_FULL = 8192
NCORES = 8
BC = B_FULL // NCORES      # 1024 batch rows per core
NG = 4                     # independent batch groups per core
BG = BC // NG              # 256 rows per group
NBT = BG // 128            # 2 batch-tiles of 128 per group
U = int(os.environ.get("K_U", "128"))  # steps per loop chunk / x window
STAGGER = os.environ.get("K_STAGGER", "1") == "1"
UNROLL = os.environ.get("K_UNROLL", "") == "1"  # python-unroll For_i (for sim)
GATE_BUFS = int(os.environ.get("K_GATE_BUFS", "0")) or 0
NO_P0 = os.environ.get("K_NO_P0", "") == "1"   # diagnostic: skip ident matmuls
NO_X = os.environ.get("K_NO_X", "") == "1"     # diagnostic: skip x projection
XB = int(os.environ.get("K_XB", "8"))          # out-projection burst size (steps)
CADD_POOL = os.environ.get("K_CADD_POOL", "") == "1"  # c'=2v+t2 on Pool vs DVE
# pair-shared x windows (1 PSUM bank per 2 groups) freeing banks for a 3rd
# gates buffer; requires U=64 so a pair window fits one bank
XPAIR = os.environ.get("K_XPAIR", "") == "1"


def _build_nc(t_total=T):
    nc = bacc.Bacc("TRN2", target_bir_lowering=False)

    # per-group contiguous: [hid, group, gate, batch-in-group]
    p0_d = nc.dram_tensor("p0", [HIDDEN, NG, 4, BG], f32r, kind="ExternalInput")
    pf_d = nc.dram_tensor("pf", [4, HIDDEN, BC], f32, kind="ExternalInput")
    weff_d = nc.dram_tensor("weff", [HIDDEN, 4 * HIDDEN], f32r, kind="ExternalInput")
    woutT_d = nc.dram_tensor("woutT", [HIDDEN, INPUT_SIZE], f32r, kind="ExternalInput")
    ident_d = nc.dram_tensor("ident", [HIDDEN, HIDDEN], f32r, kind="ExternalInput")
    ones_d = nc.dram_tensor("ones", [1, HIDDEN], f32r, kind="ExternalInput")
    brow_d = nc.dram_tensor("brow", [1, 512], f32r, kind="ExternalInput")
    y_d = nc.dram_tensor("y", [BC, t_total, INPUT_SIZE], f32, kind="ExternalOutput")
    # [p, k, t, j] view of y: batch row b = k*128 + p
    y_v = y_d.ap().rearrange("(k p) t j -> p k t j", p=128)

    gate_bufs = GATE_BUFS or (3 if XPAIR else 2)
    u0 = min(U, t_total)
    n_chunks = t_total // U if t_total >= U else 1

    with tile.TileContext(nc) as tc:
        with (
            tc.tile_pool(name="consts", bufs=1) as consts,
            tc.tile_pool(name="hpool", bufs=2) as hpool,
            tc.tile_pool(name="cpool", bufs=2 * NG) as cpool,
            tc.tile_pool(name="cell", bufs=2 * NG) as cell,
            tc.tile_pool(name="ps_gate", bufs=gate_bufs, space="PSUM") as ps_gate,
            tc.tile_pool(name="ps_x", bufs=(NG // 2 if XPAIR else NG),
                         space="PSUM") as ps_x,
            tc.tile_pool(name="xsb", bufs=2) as xsb,
        ):
            # ---- constants ----
            weff = consts.tile([HIDDEN, 4 * HIDDEN], f32r)
            woutT = consts.tile([HIDDEN, INPUT_SIZE], f32r)
            ident = consts.tile([HIDDEN, HIDDEN], f32r)
            ones = consts.tile([1, HIDDEN], f32r)
            brow = consts.tile([1, 512], f32r)
            p0 = consts.tile([128, NG, 4, BG], f32r)  # [hid, group, gate, b-in-g]
            nc.gpsimd.dma_start(out=weff, in_=weff_d[:, :])
            nc.gpsimd.dma_start(out=woutT, in_=woutT_d[:, :])
            nc.gpsimd.dma_start(out=ident, in_=ident_d[:, :])
            nc.gpsimd.dma_start(out=ones, in_=ones_d[:, :])
            nc.gpsimd.dma_start(out=brow, in_=brow_d[:, :])
            nc.gpsimd.dma_start(out=p0, in_=p0_d.ap())

            def group_phase1(gi, h_prev, c_prev, c_new, first):
                """Gates + sigmoid + c-update for group gi. Returns s."""
                bs = slice(gi * BG, (gi + 1) * BG)
                if first:
                    gates = cell.tile([128, 4, BG], f32, tag="gates_f")
                    nc.sync.dma_start(
                        out=gates,
                        in_=pf_d.ap().rearrange("g p b -> p g b")[:, :, bs],
                    )
                else:
                    gates = ps_gate.tile([128, 4, BG], f32)
                    for g in range(4):
                        nc.tensor.matmul(
                            gates[:, g, :],
                            weff[:, g * 128:(g + 1) * 128],
                            h_prev[:, :],
                            start=True, stop=NO_P0,
                        )
                        if not NO_P0:
                            nc.tensor.matmul(
                                gates[:, g, :], ident, p0[:, gi, g, :],
                                start=False, stop=True,
                            )

                s = cell.tile([128, 4, BG], f32, tag="s")
                nc.scalar.activation(s, gates, AF.Sigmoid)

                # t2 = s[f]*c on Pool ; v = (s[g]-0.5)*s[i] on DVE
                t2 = cell.tile([128, BG], f32, tag="t2")
                nc.gpsimd.tensor_mul(t2, s[:, 1, :], c_prev)
                v = cell.tile([128, BG], f32, tag="v")
                nc.vector.scalar_tensor_tensor(
                    v, s[:, 2, :], 0.5, s[:, 0, :],
                    op0=ALU.subtract, op1=ALU.mult)
                if CADD_POOL:
                    # c-state is c/2: ct' = v + sf*ct ; sigma_c uses scale=4
                    nc.gpsimd.tensor_add(c_new, v, t2)
                else:
                    nc.vector.scalar_tensor_tensor(
                        c_new, v, 2.0, t2, op0=ALU.mult, op1=ALU.add)
                return s

            def group_phase2(gi, s, sc, ht_slot):
                """ht = (sc-0.5)*s[o] written into the group's ring buffer."""
                nc.vector.scalar_tensor_tensor(
                    ht_slot, sc, 0.5, s[:, 3, :], op0=ALU.subtract, op1=ALU.mult)

            def x_burst(gi, hbuf, x_ps, s_base, nsteps):
                """Out-projections for steps [s_base, s_base+nsteps) of group gi,
                emitted after all their ht slots are written (no PE queue
                stalls on ht waits)."""
                if NO_X:
                    return
                if XPAIR:
                    xv = x_ps.rearrange(
                        "p (h k s j) -> p h k s j", h=2, k=NBT,
                        j=INPUT_SIZE)[:, gi % 2]
                else:
                    xv = x_ps.rearrange(
                        "p (k s j) -> p k s j", k=NBT, j=INPUT_SIZE)
                for s_loc in range(nsteps):
                    for k in range(NBT):
                        nc.tensor.matmul(
                            xv[:, k, s_base + s_loc, :],
                            hbuf[:, s_loc, k * 128:(k + 1) * 128],
                            woutT,
                            start=False, stop=True, skip_group_check=True,
                        )

            def step(hs, cs, ht_slots, first=False):
                """One LSTM step; ht written into ht_slots[gi]. Returns cs."""
                new_cs = [cpool.tile([128, BG], f32, name="cnew")
                          for _ in range(NG)]
                ss = [group_phase1(gi, hs[gi], cs[gi], new_cs[gi], first)
                      for gi in range(NG)]
                for gi in range(NG):
                    sc = cell.tile([128, BG], f32, tag="sc")
                    nc.scalar.activation(sc, new_cs[gi], AF.Sigmoid,
                                         scale=4.0 if CADD_POOL else 2.0)
                    group_phase2(gi, ss[gi], sc, ht_slots[gi])
                return new_cs

            def run_steps(nsteps, hs, cs, x_tiles, first_chunk=False):
                """nsteps of the recurrence with XB-step out-proj bursts."""
                for w in range(0, nsteps, XB):
                    nb = min(XB, nsteps - w)
                    hbufs = [hpool.tile([128, XB, BG], f32r, tag=f"hb{gi}",
                                        name=f"hb{gi}")
                             for gi in range(NG)]
                    for s_loc in range(nb):
                        slots = [hbufs[gi][:, s_loc, :] for gi in range(NG)]
                        cs = step(hs, cs, slots,
                                  first=(first_chunk and w == 0 and s_loc == 0))
                        hs = slots
                    for gi in range(NG):
                        x_burst(gi, hbufs[gi], x_tiles[gi], w, nb)
                return hs, cs

            def open_window(nsteps):
                """PSUM x tiles (per group, or per pair if XPAIR),
                bias-initialized via K=1 ones-matmuls."""
                x_tiles = []
                if XPAIR:
                    glen = NBT * nsteps * INPUT_SIZE
                    for p in range(NG // 2):
                        x_ps = ps_x.tile([128, 2 * glen], f32, name="x_ps")
                        nc.tensor.matmul(
                            x_ps, ones, brow[:, 0:2 * glen],
                            start=True, stop=False, skip_group_check=True,
                        )
                        x_tiles.append(x_ps)
                    return [x_tiles[gi // 2] for gi in range(NG)]
                for gi in range(NG):
                    x_ps = ps_x.tile([128, NBT * nsteps * INPUT_SIZE], f32)
                    nc.tensor.matmul(
                        x_ps, ones, brow[:, 0:NBT * nsteps * INPUT_SIZE],
                        start=True, stop=False, skip_group_check=True,
                    )
                    x_tiles.append(x_ps)
                return x_tiles

            def close_window(x_tiles, nsteps, ts, xsb_pool):
                if XPAIR:
                    for p in range(NG // 2):
                        x_sb = xsb_pool.tile(
                            [128, 2 * NBT * nsteps * INPUT_SIZE], f32,
                            tag=f"xsb{p}")
                        nc.vector.tensor_copy(x_sb, x_tiles[2 * p])
                        xv = x_sb.rearrange(
                            "p (h k s j) -> p (h k) s j", h=2, k=NBT,
                            j=INPUT_SIZE)
                        nc.sync.dma_start(
                            out=y_v[:, 2 * NBT * p:2 * NBT * (p + 1), ts, :],
                            in_=xv)
                    return
                for gi in range(NG):
                    x_sb = xsb_pool.tile(
                        [128, NBT * nsteps * INPUT_SIZE], f32, tag=f"xsb{gi}")
                    nc.vector.tensor_copy(x_sb, x_tiles[gi])
                    xv = x_sb.rearrange(
                        "p (k s j) -> p k s j", k=NBT, j=INPUT_SIZE)
                    nc.sync.dma_start(
                        out=y_v[:, NBT * gi:NBT * (gi + 1), ts, :], in_=xv)

            # ---- peeled chunk 0 ----
            cs = []
            for gi in range(NG):
                c0 = cpool.tile([128, BG], f32, name="c0")
                nc.vector.memset(c0, 0.0)
                cs.append(c0)
            hs0 = [hpool.tile([128, XB, BG], f32r, tag=f"hb{gi}", name=f"hb{gi}")
                   for gi in range(NG)]
            hs = [hs0[gi][:, XB - 1, :] for gi in range(NG)]
            x_tiles = open_window(u0)
            hs, cs = run_steps(u0, hs, cs, x_tiles, first_chunk=True)
            close_window(x_tiles, u0, slice(0, u0), xsb)

            # ---- chunks 1..n_chunks-1 ----
            if n_chunks > 1 and UNROLL:
                for ci in range(1, n_chunks):
                    x_tiles = open_window(U)
                    hs, cs = run_steps(U, hs, cs, x_tiles)
                    close_window(x_tiles, U, slice(ci * U, (ci + 1) * U), xsb)
            elif n_chunks > 1:
                with tc.For_i(1, n_chunks, 1, staggered_reset=STAGGER) as ci:
                    x_tiles = open_window(U)
                    hs2, cs2 = run_steps(U, hs, cs, x_tiles)
                    close_window(x_tiles, U, bass.ts(ci, U), xsb)
    nc.compile()
    return nc


_NC_CACHE = {}


def _get_nc():
    if "nc" not in _NC_CACHE:
        _NC_CACHE["nc"] = _build_nc()
    return _NC_CACHE["nc"]


def kernel(z, condition, start_point, W_ih, W_hh, b_ih, b_hh, W_out, b_out, seq_len):
    z = np.asarray(z, dtype=np.float32)
    condition = np.asarray(condition, dtype=np.float32)
    start_point = np.asarray(start_point, dtype=np.float32)
    W_ih = np.asarray(W_ih, dtype=np.float32)
    W_hh = np.asarray(W_hh, dtype=np.float32)
    b_ih = np.asarray(b_ih, dtype=np.float32)
    b_hh = np.asarray(b_hh, dtype=np.float32)
    W_out = np.asarray(W_out, dtype=np.float32)
    b_out = np.asarray(b_out, dtype=np.float32)
    assert int(seq_len) == T and z.shape[0] == B_FULL

    B = z.shape[0]
    dt_col = np.full((B, 1), 0.05, dtype=np.float32)
    static_in = np.concatenate([z, condition, dt_col], axis=-1)          # [B, 37]
    static_proj = static_in @ W_ih[:, INPUT_SIZE:].T + b_ih + b_hh       # [B, 512]
    Wx = W_ih[:, :INPUT_SIZE]                                            # [512, 2]
    P0 = static_proj + b_out @ Wx.T                                      # [B, 512]
    Pf = static_proj + start_point @ Wx.T                                # [B, 512]
    W_comb = (W_hh.T + W_out.T @ Wx.T).astype(np.float32)                # [128, 512]

    # h-state is ht = h/2  ->  double W_comb and W_out.T.
    # tanh(g) = 2 sig(2g) - 1  ->  double the g-gate block again (cols 256:384
    # of W_eff, rows 256:384 of P0/Pf).
    W_eff = 2.0 * W_comb
    W_eff[:, 2 * HIDDEN:3 * HIDDEN] *= 2.0
    P0e = P0.copy()
    P0e[:, 2 * HIDDEN:3 * HIDDEN] *= 2.0
    Pfe = Pf.copy()
    Pfe[:, 2 * HIDDEN:3 * HIDDEN] *= 2.0

    # unit-major, per-gate: [4, 128, B], gate order [i, f, g, o]
    P0_t = np.ascontiguousarray(P0e.T.reshape(4, HIDDEN, B), dtype=np.float32)
    Pf_t = np.ascontiguousarray(Pfe.T.reshape(4, HIDDEN, B), dtype=np.float32)
    # per-core per-group contiguous view for p0: [hid, NG, 4, BG]
    # P0_t[g, p, b] with b = core*BC + grp*BG + r  ->  p0[p, grp, g, r]
    P0_pg = np.ascontiguousarray(
        P0_t.reshape(4, HIDDEN, NCORES, NG, BG).transpose(2, 1, 3, 0, 4))
    woutT2 = np.ascontiguousarray(2.0 * W_out.T, dtype=np.float32)       # [128, 2]
    ident = np.eye(HIDDEN, dtype=np.float32)
    ones_row = np.ones((1, HIDDEN), dtype=np.float32)
    brow = np.tile(b_out, 256)[None, :].astype(np.float32)               # [1, 512]

    nc = _get_nc()
    in_maps = []
    for c in range(NCORES):
        bs = slice(c * BC, (c + 1) * BC)
        in_maps.append({
            "p0": P0_pg[c],
            "pf": np.ascontiguousarray(Pf_t[:, :, bs]),
            "weff": np.ascontiguousarray(W_eff),
            "woutT": woutT2,
            "ident": ident,
            "ones": ones_row,
            "brow": brow,
        })
    global _last_in_maps
    _last_in_maps = in_maps
    res = run_bass_kernel_spmd(nc, in_maps, core_ids=list(range(NCORES)))
    out = np.concatenate([r["y"] for r in res.results], axis=0)
    return out


_last_in_maps = None


# revision 35
# speedup vs baseline: 1.0735x; 1.0735x over previous
"""Trainium2 Bass kernel for nn_CVAE decoder LSTM.

Data-parallel over 8 NeuronCores: batch 8192 -> 1024 per core.

Math (per core, B=1024, T=2048, H=128). Reference step:
    gates = x_t @ Wx.T + static_proj + h @ W_hh.T
    i,f,g,o = split(gates); c' = sig(f)*c + sig(i)*tanh(g)
    h' = sig(o)*tanh(c'); x' = h' @ W_out.T + b_out; ys[t] = x'
Folds applied (all exact):
  1. x_t = h_t @ W_out.T + b_out for t>=1:
       W_comb = W_hh.T + W_out.T @ Wx.T        [128, 512]
       P0     = static_proj + b_out @ Wx.T     [B, 512]
       gates_t = h_t @ W_comb + P0   (t>=1);  gates_0 = Pf (h_0 = 0)
  2. tanh(x) = 2*sigmoid(2x) - 1: double the g-gate columns of W_comb/P0 so
     ONE sigmoid instruction covers all 4 gates; the affine 2s-1 rides the
     scalar_tensor_tensor multiplies on DVE.
  3. h = sig(o)*tanh(c) = 2*(sig(2c)-0.5)*sig(o) =: 2*ht. The x2 is folded
     into W_comb and W_out (both doubled), so the kernel state is
     ht = (sig(2c)-0.5)*sig(o), one STT op.
Per group (BG=256) per step:
  PE:   4 gate matmuls (W_eff) + 4 identity matmuls (P0 accumulate)
  ACT:  s = sigmoid(gates) over [128,4,BG] (ONE instr); sc = sigmoid(2c')
  Pool: t2 = s[f]*c
  DVE:  v = (s[g]-0.5)*s[i] ; c' = 2v + t2 ; ht = (sc-0.5)*s[o]
  PE:   x-projection into a per-group PSUM window tile [128,2,U,2]
        (bias pre-filled via a K=1 ones-matmul), emitted in XB=8-step
        BURSTS after the ht ring-buffer slots are all written -- a
        per-step out-proj would park an LDWEIGHTS waiting on ht at the
        PE queue head and stall the other groups' gate matmuls behind
        it (measured +2.5 ms). Window copied PSUM->SBUF, DMA'd per U.
Batch is split into NG=4 independent groups of BG=256 whose serial cell
chains stagger across engines. Chain: PE -> ACT -> {Pool,DVE} -> DVE ->
ACT -> DVE -> PE. ACT is the throughput bound (sim: 99.6% busy); NG=2
halves ACT instr overhead but the longer per-group chain latency loses
more (measured), and merging sigmoid_c across groups couples chains
(also measured worse). Failed variants (measured slower): c-add on Pool
(17.3ms), XB=4 (15.9ms), no-P0-ident (17.9ms), sc pair-merge (sim 15.1),
XPAIR pair-shared x windows + 3 gate bufs at U=64 (20.3ms -- bank-overlap
serialization of the paired groups' PSUM writes).
"""

import os
import numpy as np

import concourse.bass as bass
import concourse.bacc as bacc
import concourse.tile as tile
from concourse import mybir
from concourse.bass_utils import run_bass_kernel_spmd

f32 = mybir.dt.float32
f32r = mybir.dt.float32r
AF = mybir.ActivationFunctionType
ALU = mybir.AluOpType

HIDDEN = 128
INPUT_SIZE = 2
T = 2048
B_FULL = 8192
NCORES = 8
BC = B_FULL // NCORES      # 1024 batch rows per core
NG = 4                     # independent batch groups per core
BG = BC // NG              # 256 rows per group
NBT = BG // 128            # 2 batch-tiles of 128 per group
U = int(os.environ.get("K_U", "128"))  # steps per loop chunk / x window
STAGGER = os.environ.get("K_STAGGER", "1") == "1"
UNROLL = os.environ.get("K_UNROLL", "") == "1"  # python-unroll For_i (for sim)
GATE_BUFS = int(os.environ.get("K_GATE_BUFS", "0")) or 0
NO_P0 = os.environ.get("K_NO_P0", "") == "1"   # diagnostic: skip ident matmuls
NO_X = os.environ.get("K_NO_X", "") == "1"     # diagnostic: skip x projection
XB = int(os.environ.get("K_XB", "8"))          # out-projection burst size (steps)
CADD_POOL = os.environ.get("K_CADD_POOL", "") == "1"  # c'=2v+t2 on Pool vs DVE
# pair-shared x windows (1 PSUM bank per 2 groups) freeing banks for a 3rd
# gates buffer; requires U=64 so a pair window fits one bank
XPAIR = os.environ.get("K_XPAIR", "") == "1"
ILV = os.environ.get("K_ILV", "1") == "1"      # interleave sc/phase2 per group
HB = int(os.environ.get("K_HB", "2"))          # ht ring buffers


def _build_nc(t_total=T):
    nc = bacc.Bacc("TRN2", target_bir_lowering=False)

    # per-group contiguous: [hid, group, gate, batch-in-group]
    p0_d = nc.dram_tensor("p0", [HIDDEN, NG, 4, BG], f32r, kind="ExternalInput")
    pf_d = nc.dram_tensor("pf", [4, HIDDEN, BC], f32, kind="ExternalInput")
    weff_d = nc.dram_tensor("weff", [HIDDEN, 4 * HIDDEN], f32r, kind="ExternalInput")
    woutT_d = nc.dram_tensor("woutT", [HIDDEN, INPUT_SIZE], f32r, kind="ExternalInput")
    ident_d = nc.dram_tensor("ident", [HIDDEN, HIDDEN], f32r, kind="ExternalInput")
    ones_d = nc.dram_tensor("ones", [1, HIDDEN], f32r, kind="ExternalInput")
    brow_d = nc.dram_tensor("brow", [1, 512], f32r, kind="ExternalInput")
    y_d = nc.dram_tensor("y", [BC, t_total, INPUT_SIZE], f32, kind="ExternalOutput")
    # [p, k, t, j] view of y: batch row b = k*128 + p
    y_v = y_d.ap().rearrange("(k p) t j -> p k t j", p=128)

    gate_bufs = GATE_BUFS or (3 if XPAIR else 2)
    u0 = min(U, t_total)
    n_chunks = t_total // U if t_total >= U else 1

    with tile.TileContext(nc) as tc:
        with (
            tc.tile_pool(name="consts", bufs=1) as consts,
            tc.tile_pool(name="hpool", bufs=HB) as hpool,
            tc.tile_pool(name="cpool", bufs=2 * NG) as cpool,
            tc.tile_pool(name="cell", bufs=2 * NG) as cell,
            tc.tile_pool(name="ps_gate", bufs=gate_bufs, space="PSUM") as ps_gate,
            tc.tile_pool(name="ps_x", bufs=(NG // 2 if XPAIR else NG),
                         space="PSUM") as ps_x,
            tc.tile_pool(name="xsb", bufs=2) as xsb,
        ):
            # ---- constants ----
            weff = consts.tile([HIDDEN, 4 * HIDDEN], f32r)
            woutT = consts.tile([HIDDEN, INPUT_SIZE], f32r)
            ident = consts.tile([HIDDEN, HIDDEN], f32r)
            ones = consts.tile([1, HIDDEN], f32r)
            brow = consts.tile([1, 512], f32r)
            p0 = consts.tile([128, NG, 4, BG], f32r)  # [hid, group, gate, b-in-g]
            nc.gpsimd.dma_start(out=weff, in_=weff_d[:, :])
            nc.gpsimd.dma_start(out=woutT, in_=woutT_d[:, :])
            nc.gpsimd.dma_start(out=ident, in_=ident_d[:, :])
            nc.gpsimd.dma_start(out=ones, in_=ones_d[:, :])
            nc.gpsimd.dma_start(out=brow, in_=brow_d[:, :])
            nc.gpsimd.dma_start(out=p0, in_=p0_d.ap())

            def group_phase1(gi, h_prev, c_prev, c_new, first):
                """Gates + sigmoid + c-update for group gi. Returns s."""
                bs = slice(gi * BG, (gi + 1) * BG)
                if first:
                    gates = cell.tile([128, 4, BG], f32, tag="gates_f")
                    nc.sync.dma_start(
                        out=gates,
                        in_=pf_d.ap().rearrange("g p b -> p g b")[:, :, bs],
                    )
                else:
                    gates = ps_gate.tile([128, 4, BG], f32)
                    for g in range(4):
                        nc.tensor.matmul(
                            gates[:, g, :],
                            weff[:, g * 128:(g + 1) * 128],
                            h_prev[:, :],
                            start=True, stop=NO_P0,
                        )
                        if not NO_P0:
                            nc.tensor.matmul(
                                gates[:, g, :], ident, p0[:, gi, g, :],
                                start=False, stop=True,
                            )

                s = cell.tile([128, 4, BG], f32, tag="s")
                nc.scalar.activation(s, gates, AF.Sigmoid)

                # t2 = s[f]*c on Pool ; v = (s[g]-0.5)*s[i] on DVE
                t2 = cell.tile([128, BG], f32, tag="t2")
                nc.gpsimd.tensor_mul(t2, s[:, 1, :], c_prev)
                v = cell.tile([128, BG], f32, tag="v")
                nc.vector.scalar_tensor_tensor(
                    v, s[:, 2, :], 0.5, s[:, 0, :],
                    op0=ALU.subtract, op1=ALU.mult)
                if CADD_POOL:
                    # c-state is c/2: ct' = v + sf*ct ; sigma_c uses scale=4
                    nc.gpsimd.tensor_add(c_new, v, t2)
                else:
                    nc.vector.scalar_tensor_tensor(
                        c_new, v, 2.0, t2, op0=ALU.mult, op1=ALU.add)
                return s

            def group_phase2(gi, s, sc, ht_slot):
                """ht = (sc-0.5)*s[o] written into the group's ring buffer."""
                nc.vector.scalar_tensor_tensor(
                    ht_slot, sc, 0.5, s[:, 3, :], op0=ALU.subtract, op1=ALU.mult)

            def x_burst(gi, hbuf, x_ps, s_base, nsteps):
                """Out-projections for steps [s_base, s_base+nsteps) of group gi,
                emitted after all their ht slots are written (no PE queue
                stalls on ht waits)."""
                if NO_X:
                    return
                if XPAIR:
                    xv = x_ps.rearrange(
                        "p (h k s j) -> p h k s j", h=2, k=NBT,
                        j=INPUT_SIZE)[:, gi % 2]
                else:
                    xv = x_ps.rearrange(
                        "p (k s j) -> p k s j", k=NBT, j=INPUT_SIZE)
                for s_loc in range(nsteps):
                    for k in range(NBT):
                        nc.tensor.matmul(
                            xv[:, k, s_base + s_loc, :],
                            hbuf[:, s_loc, k * 128:(k + 1) * 128],
                            woutT,
                            start=False, stop=True, skip_group_check=True,
                        )

            def step(hs, cs, ht_slots, first=False):
                """One LSTM step; ht written into ht_slots[gi]. Returns cs."""
                new_cs = [cpool.tile([128, BG], f32, name="cnew")
                          for _ in range(NG)]
                if ILV:
                    for gi in range(NG):
                        s = group_phase1(gi, hs[gi], cs[gi], new_cs[gi], first)
                        sc = cell.tile([128, BG], f32, tag="sc")
                        nc.scalar.activation(sc, new_cs[gi], AF.Sigmoid,
                                             scale=4.0 if CADD_POOL else 2.0)
                        group_phase2(gi, s, sc, ht_slots[gi])
                    return new_cs
                ss = [group_phase1(gi, hs[gi], cs[gi], new_cs[gi], first)
                      for gi in range(NG)]
                for gi in range(NG):
                    sc = cell.tile([128, BG], f32, tag="sc")
                    nc.scalar.activation(sc, new_cs[gi], AF.Sigmoid,
                                         scale=4.0 if CADD_POOL else 2.0)
                    group_phase2(gi, ss[gi], sc, ht_slots[gi])
                return new_cs

            def run_steps(nsteps, hs, cs, x_tiles, first_chunk=False):
                """nsteps of the recurrence with XB-step out-proj bursts."""
                for w in range(0, nsteps, XB):
                    nb = min(XB, nsteps - w)
                    hbufs = [hpool.tile([128, XB, BG], f32r, tag=f"hb{gi}",
                                        name=f"hb{gi}")
                             for gi in range(NG)]
                    for s_loc in range(nb):
                        slots = [hbufs[gi][:, s_loc, :] for gi in range(NG)]
                        cs = step(hs, cs, slots,
                                  first=(first_chunk and w == 0 and s_loc == 0))
                        hs = slots
                    for gi in range(NG):
                        x_burst(gi, hbufs[gi], x_tiles[gi], w, nb)
                return hs, cs

            def open_window(nsteps):
                """PSUM x tiles (per group, or per pair if XPAIR),
                bias-initialized via K=1 ones-matmuls."""
                x_tiles = []
                if XPAIR:
                    glen = NBT * nsteps * INPUT_SIZE
                    for p in range(NG // 2):
                        x_ps = ps_x.tile([128, 2 * glen], f32, name="x_ps")
                        nc.tensor.matmul(
                            x_ps, ones, brow[:, 0:2 * glen],
                            start=True, stop=False, skip_group_check=True,
                        )
                        x_tiles.append(x_ps)
                    return [x_tiles[gi // 2] for gi in range(NG)]
                for gi in range(NG):
                    x_ps = ps_x.tile([128, NBT * nsteps * INPUT_SIZE], f32)
                    nc.tensor.matmul(
                        x_ps, ones, brow[:, 0:NBT * nsteps * INPUT_SIZE],
                        start=True, stop=False, skip_group_check=True,
                    )
                    x_tiles.append(x_ps)
                return x_tiles

            def close_window(x_tiles, nsteps, ts, xsb_pool):
                if XPAIR:
                    for p in range(NG // 2):
                        x_sb = xsb_pool.tile(
                            [128, 2 * NBT * nsteps * INPUT_SIZE], f32,
                            tag=f"xsb{p}")
                        nc.vector.tensor_copy(x_sb, x_tiles[2 * p])
                        xv = x_sb.rearrange(
                            "p (h k s j) -> p (h k) s j", h=2, k=NBT,
                            j=INPUT_SIZE)
                        nc.sync.dma_start(
                            out=y_v[:, 2 * NBT * p:2 * NBT * (p + 1), ts, :],
                            in_=xv)
                    return
                for gi in range(NG):
                    x_sb = xsb_pool.tile(
                        [128, NBT * nsteps * INPUT_SIZE], f32, tag=f"xsb{gi}")
                    nc.vector.tensor_copy(x_sb, x_tiles[gi])
                    xv = x_sb.rearrange(
                        "p (k s j) -> p k s j", k=NBT, j=INPUT_SIZE)
                    nc.sync.dma_start(
                        out=y_v[:, NBT * gi:NBT * (gi + 1), ts, :], in_=xv)

            # ---- peeled chunk 0 ----
            cs = []
            for gi in range(NG):
                c0 = cpool.tile([128, BG], f32, name="c0")
                nc.vector.memset(c0, 0.0)
                cs.append(c0)
            hs0 = [hpool.tile([128, XB, BG], f32r, tag=f"hb{gi}", name=f"hb{gi}")
                   for gi in range(NG)]
            hs = [hs0[gi][:, XB - 1, :] for gi in range(NG)]
            x_tiles = open_window(u0)
            hs, cs = run_steps(u0, hs, cs, x_tiles, first_chunk=True)
            close_window(x_tiles, u0, slice(0, u0), xsb)

            # ---- chunks 1..n_chunks-1 ----
            if n_chunks > 1 and UNROLL:
                for ci in range(1, n_chunks):
                    x_tiles = open_window(U)
                    hs, cs = run_steps(U, hs, cs, x_tiles)
                    close_window(x_tiles, U, slice(ci * U, (ci + 1) * U), xsb)
            elif n_chunks > 1:
                with tc.For_i(1, n_chunks, 1, staggered_reset=STAGGER) as ci:
                    x_tiles = open_window(U)
                    hs2, cs2 = run_steps(U, hs, cs, x_tiles)
                    close_window(x_tiles, U, bass.ts(ci, U), xsb)
    nc.compile()
    return nc


_NC_CACHE = {}


def _get_nc():
    if "nc" not in _NC_CACHE:
        _NC_CACHE["nc"] = _build_nc()
    return _NC_CACHE["nc"]


def kernel(z, condition, start_point, W_ih, W_hh, b_ih, b_hh, W_out, b_out, seq_len):
    z = np.asarray(z, dtype=np.float32)
    condition = np.asarray(condition, dtype=np.float32)
    start_point = np.asarray(start_point, dtype=np.float32)
    W_ih = np.asarray(W_ih, dtype=np.float32)
    W_hh = np.asarray(W_hh, dtype=np.float32)
    b_ih = np.asarray(b_ih, dtype=np.float32)
    b_hh = np.asarray(b_hh, dtype=np.float32)
    W_out = np.asarray(W_out, dtype=np.float32)
    b_out = np.asarray(b_out, dtype=np.float32)
    assert int(seq_len) == T and z.shape[0] == B_FULL

    B = z.shape[0]
    dt_col = np.full((B, 1), 0.05, dtype=np.float32)
    static_in = np.concatenate([z, condition, dt_col], axis=-1)          # [B, 37]
    static_proj = static_in @ W_ih[:, INPUT_SIZE:].T + b_ih + b_hh       # [B, 512]
    Wx = W_ih[:, :INPUT_SIZE]                                            # [512, 2]
    P0 = static_proj + b_out @ Wx.T                                      # [B, 512]
    Pf = static_proj + start_point @ Wx.T                                # [B, 512]
    W_comb = (W_hh.T + W_out.T @ Wx.T).astype(np.float32)                # [128, 512]

    # h-state is ht = h/2  ->  double W_comb and W_out.T.
    # tanh(g) = 2 sig(2g) - 1  ->  double the g-gate block again (cols 256:384
    # of W_eff, rows 256:384 of P0/Pf).
    W_eff = 2.0 * W_comb
    W_eff[:, 2 * HIDDEN:3 * HIDDEN] *= 2.0
    P0e = P0.copy()
    P0e[:, 2 * HIDDEN:3 * HIDDEN] *= 2.0
    Pfe = Pf.copy()
    Pfe[:, 2 * HIDDEN:3 * HIDDEN] *= 2.0

    # unit-major, per-gate: [4, 128, B], gate order [i, f, g, o]
    P0_t = np.ascontiguousarray(P0e.T.reshape(4, HIDDEN, B), dtype=np.float32)
    Pf_t = np.ascontiguousarray(Pfe.T.reshape(4, HIDDEN, B), dtype=np.float32)
    # per-core per-group contiguous view for p0: [hid, NG, 4, BG]
    # P0_t[g, p, b] with b = core*BC + grp*BG + r  ->  p0[p, grp, g, r]
    P0_pg = np.ascontiguousarray(
        P0_t.reshape(4, HIDDEN, NCORES, NG, BG).transpose(2, 1, 3, 0, 4))
    woutT2 = np.ascontiguousarray(2.0 * W_out.T, dtype=np.float32)       # [128, 2]
    ident = np.eye(HIDDEN, dtype=np.float32)
    ones_row = np.ones((1, HIDDEN), dtype=np.float32)
    brow = np.tile(b_out, 256)[None, :].astype(np.float32)               # [1, 512]

    nc = _get_nc()
    in_maps = []
    for c in range(NCORES):
        bs = slice(c * BC, (c + 1) * BC)
        in_maps.append({
            "p0": P0_pg[c],
            "pf": np.ascontiguousarray(Pf_t[:, :, bs]),
            "weff": np.ascontiguousarray(W_eff),
            "woutT": woutT2,
            "ident": ident,
            "ones": ones_row,
            "brow": brow,
        })
    global _last_in_maps
    _last_in_maps = in_maps
    res = run_bass_kernel_spmd(nc, in_maps, core_ids=list(range(NCORES)))
    out = np.concatenate([r["y"] for r in res.results], axis=0)
    return out


_last_in_maps = None


# revision 37
# speedup vs baseline: 1.0757x; 1.0020x over previous
"""Trainium2 Bass kernel for nn_CVAE decoder LSTM.

Data-parallel over 8 NeuronCores: batch 8192 -> 1024 per core.

Math (per core, B=1024, T=2048, H=128). Reference step:
    gates = x_t @ Wx.T + static_proj + h @ W_hh.T
    i,f,g,o = split(gates); c' = sig(f)*c + sig(i)*tanh(g)
    h' = sig(o)*tanh(c'); x' = h' @ W_out.T + b_out; ys[t] = x'
Folds applied (all exact):
  1. x_t = h_t @ W_out.T + b_out for t>=1:
       W_comb = W_hh.T + W_out.T @ Wx.T        [128, 512]
       P0     = static_proj + b_out @ Wx.T     [B, 512]
       gates_t = h_t @ W_comb + P0   (t>=1);  gates_0 = Pf (h_0 = 0)
  2. tanh(x) = 2*sigmoid(2x) - 1: double the g-gate columns of W_comb/P0 so
     ONE sigmoid instruction covers all 4 gates; the affine 2s-1 rides the
     scalar_tensor_tensor multiplies on DVE.
  3. h = sig(o)*tanh(c) = 2*(sig(2c)-0.5)*sig(o) =: 2*ht. The x2 is folded
     into W_comb and W_out (both doubled), so the kernel state is
     ht = (sig(2c)-0.5)*sig(o), one STT op.
Per group (BG=256) per step:
  PE:   4 gate matmuls (W_eff) + 4 identity matmuls (P0 accumulate)
  ACT:  s = sigmoid(gates) over [128,4,BG] (ONE instr); sc = sigmoid(2c')
  Pool: t2 = s[f]*c
  DVE:  v = (s[g]-0.5)*s[i] ; c' = 2v + t2 ; ht = (sc-0.5)*s[o]
  PE:   x-projection into a per-group PSUM window tile [128,2,U,2]
        (bias pre-filled via a K=1 ones-matmul), emitted in XB=8-step
        BURSTS after the ht ring-buffer slots are all written -- a
        per-step out-proj would park an LDWEIGHTS waiting on ht at the
        PE queue head and stall the other groups' gate matmuls behind
        it (measured +2.5 ms). Window copied PSUM->SBUF, DMA'd per U.
Batch is split into NG=4 independent groups of BG=256 whose serial cell
chains stagger across engines. Chain: PE -> ACT -> {Pool,DVE} -> DVE ->
ACT -> DVE -> PE. ACT is the throughput bound (sim: 99.6% busy); NG=2
halves ACT instr overhead but the longer per-group chain latency loses
more (measured), and merging sigmoid_c across groups couples chains
(also measured worse). Failed variants (measured slower): c-add on Pool
(17.3ms), XB=4 (15.9ms), no-P0-ident (17.9ms), sc pair-merge (sim 15.1),
XPAIR pair-shared x windows + 3 gate bufs at U=64 (20.3ms -- bank-overlap
serialization of the paired groups' PSUM writes).
Emission order matters: interleaving each group's sigmoid_c/ht emission
immediately after its phase1 (ILV=1, default) measured 13.7ms vs 14.7ms
for phase-grouped emission -- same instructions and deps, different
TileScheduler priority order.
"""

import os
import numpy as np

import concourse.bass as bass
import concourse.bacc as bacc
import concourse.tile as tile
from concourse import mybir
from concourse.bass_utils import run_bass_kernel_spmd

f32 = mybir.dt.float32
f32r = mybir.dt.float32r
AF = mybir.ActivationFunctionType
ALU = mybir.AluOpType

HIDDEN = 128
INPUT_SIZE = 2
T = 2048
B_FULL = 8192
NCORES = 8
BC = B_FULL // NCORES      # 1024 batch rows per core
NG = 4                     # independent batch groups per core
BG = BC // NG              # 256 rows per group
NBT = BG // 128            # 2 batch-tiles of 128 per group
U = int(os.environ.get("K_U", "128"))  # steps per loop chunk / x window
STAGGER = os.environ.get("K_STAGGER", "1") == "1"
UNROLL = os.environ.get("K_UNROLL", "") == "1"  # python-unroll For_i (for sim)
GATE_BUFS = int(os.environ.get("K_GATE_BUFS", "0")) or 0
NO_P0 = os.environ.get("K_NO_P0", "") == "1"   # diagnostic: skip ident matmuls
NO_X = os.environ.get("K_NO_X", "") == "1"     # diagnostic: skip x projection
XB = int(os.environ.get("K_XB", "8"))          # out-projection burst size (steps)
CADD_POOL = os.environ.get("K_CADD_POOL", "") == "1"  # c'=2v+t2 on Pool vs DVE
# pair-shared x windows (1 PSUM bank per 2 groups) freeing banks for a 3rd
# gates buffer; requires U=64 so a pair window fits one bank
XPAIR = os.environ.get("K_XPAIR", "") == "1"
ILV = os.environ.get("K_ILV", "1") == "1"      # interleave sc/phase2 per group
HB = int(os.environ.get("K_HB", "2"))          # ht ring buffers
ROT = os.environ.get("K_ROT", "") == "1"       # rotate group emission order


def _build_nc(t_total=T):
    nc = bacc.Bacc("TRN2", target_bir_lowering=False)

    # per-group contiguous: [hid, group, gate, batch-in-group]
    p0_d = nc.dram_tensor("p0", [HIDDEN, NG, 4, BG], f32r, kind="ExternalInput")
    pf_d = nc.dram_tensor("pf", [4, HIDDEN, BC], f32, kind="ExternalInput")
    weff_d = nc.dram_tensor("weff", [HIDDEN, 4 * HIDDEN], f32r, kind="ExternalInput")
    woutT_d = nc.dram_tensor("woutT", [HIDDEN, INPUT_SIZE], f32r, kind="ExternalInput")
    ident_d = nc.dram_tensor("ident", [HIDDEN, HIDDEN], f32r, kind="ExternalInput")
    ones_d = nc.dram_tensor("ones", [1, HIDDEN], f32r, kind="ExternalInput")
    brow_d = nc.dram_tensor("brow", [1, 512], f32r, kind="ExternalInput")
    y_d = nc.dram_tensor("y", [BC, t_total, INPUT_SIZE], f32, kind="ExternalOutput")
    # [p, k, t, j] view of y: batch row b = k*128 + p
    y_v = y_d.ap().rearrange("(k p) t j -> p k t j", p=128)

    gate_bufs = GATE_BUFS or (3 if XPAIR else 2)
    u0 = min(U, t_total)
    n_chunks = t_total // U if t_total >= U else 1

    with tile.TileContext(nc) as tc:
        with (
            tc.tile_pool(name="consts", bufs=1) as consts,
            tc.tile_pool(name="hpool", bufs=HB) as hpool,
            tc.tile_pool(name="cpool", bufs=2 * NG) as cpool,
            tc.tile_pool(name="cell", bufs=2 * NG) as cell,
            tc.tile_pool(name="ps_gate", bufs=gate_bufs, space="PSUM") as ps_gate,
            tc.tile_pool(name="ps_x", bufs=(NG // 2 if XPAIR else NG),
                         space="PSUM") as ps_x,
            tc.tile_pool(name="xsb", bufs=2) as xsb,
        ):
            # ---- constants ----
            weff = consts.tile([HIDDEN, 4 * HIDDEN], f32r)
            woutT = consts.tile([HIDDEN, INPUT_SIZE], f32r)
            ident = consts.tile([HIDDEN, HIDDEN], f32r)
            ones = consts.tile([1, HIDDEN], f32r)
            brow = consts.tile([1, 512], f32r)
            p0 = consts.tile([128, NG, 4, BG], f32r)  # [hid, group, gate, b-in-g]
            nc.gpsimd.dma_start(out=weff, in_=weff_d[:, :])
            nc.gpsimd.dma_start(out=woutT, in_=woutT_d[:, :])
            nc.gpsimd.dma_start(out=ident, in_=ident_d[:, :])
            nc.gpsimd.dma_start(out=ones, in_=ones_d[:, :])
            nc.gpsimd.dma_start(out=brow, in_=brow_d[:, :])
            nc.gpsimd.dma_start(out=p0, in_=p0_d.ap())

            def group_phase1(gi, h_prev, c_prev, c_new, first):
                """Gates + sigmoid + c-update for group gi. Returns s."""
                bs = slice(gi * BG, (gi + 1) * BG)
                if first:
                    gates = cell.tile([128, 4, BG], f32, tag="gates_f")
                    nc.sync.dma_start(
                        out=gates,
                        in_=pf_d.ap().rearrange("g p b -> p g b")[:, :, bs],
                    )
                else:
                    gates = ps_gate.tile([128, 4, BG], f32)
                    for g in range(4):
                        nc.tensor.matmul(
                            gates[:, g, :],
                            weff[:, g * 128:(g + 1) * 128],
                            h_prev[:, :],
                            start=True, stop=NO_P0,
                        )
                        if not NO_P0:
                            nc.tensor.matmul(
                                gates[:, g, :], ident, p0[:, gi, g, :],
                                start=False, stop=True,
                            )

                s = cell.tile([128, 4, BG], f32, tag="s")
                nc.scalar.activation(s, gates, AF.Sigmoid)

                # t2 = s[f]*c on Pool ; v = (s[g]-0.5)*s[i] on DVE
                t2 = cell.tile([128, BG], f32, tag="t2")
                nc.gpsimd.tensor_mul(t2, s[:, 1, :], c_prev)
                v = cell.tile([128, BG], f32, tag="v")
                nc.vector.scalar_tensor_tensor(
                    v, s[:, 2, :], 0.5, s[:, 0, :],
                    op0=ALU.subtract, op1=ALU.mult)
                if CADD_POOL:
                    # c-state is c/2: ct' = v + sf*ct ; sigma_c uses scale=4
                    nc.gpsimd.tensor_add(c_new, v, t2)
                else:
                    nc.vector.scalar_tensor_tensor(
                        c_new, v, 2.0, t2, op0=ALU.mult, op1=ALU.add)
                return s

            def group_phase2(gi, s, sc, ht_slot):
                """ht = (sc-0.5)*s[o] written into the group's ring buffer."""
                nc.vector.scalar_tensor_tensor(
                    ht_slot, sc, 0.5, s[:, 3, :], op0=ALU.subtract, op1=ALU.mult)

            def x_burst(gi, hbuf, x_ps, s_base, nsteps):
                """Out-projections for steps [s_base, s_base+nsteps) of group gi,
                emitted after all their ht slots are written (no PE queue
                stalls on ht waits)."""
                if NO_X:
                    return
                if XPAIR:
                    xv = x_ps.rearrange(
                        "p (h k s j) -> p h k s j", h=2, k=NBT,
                        j=INPUT_SIZE)[:, gi % 2]
                else:
                    xv = x_ps.rearrange(
                        "p (k s j) -> p k s j", k=NBT, j=INPUT_SIZE)
                for s_loc in range(nsteps):
                    for k in range(NBT):
                        nc.tensor.matmul(
                            xv[:, k, s_base + s_loc, :],
                            hbuf[:, s_loc, k * 128:(k + 1) * 128],
                            woutT,
                            start=False, stop=True, skip_group_check=True,
                        )

            def step(hs, cs, ht_slots, first=False, rot=0):
                """One LSTM step; ht written into ht_slots[gi]. Returns cs."""
                new_cs = [cpool.tile([128, BG], f32, name="cnew")
                          for _ in range(NG)]
                order = [(g + rot) % NG for g in range(NG)] if ROT else range(NG)
                if ILV:
                    for gi in order:
                        s = group_phase1(gi, hs[gi], cs[gi], new_cs[gi], first)
                        sc = cell.tile([128, BG], f32, tag="sc")
                        nc.scalar.activation(sc, new_cs[gi], AF.Sigmoid,
                                             scale=4.0 if CADD_POOL else 2.0)
                        group_phase2(gi, s, sc, ht_slots[gi])
                    return new_cs
                ss = [group_phase1(gi, hs[gi], cs[gi], new_cs[gi], first)
                      for gi in range(NG)]
                for gi in range(NG):
                    sc = cell.tile([128, BG], f32, tag="sc")
                    nc.scalar.activation(sc, new_cs[gi], AF.Sigmoid,
                                         scale=4.0 if CADD_POOL else 2.0)
                    group_phase2(gi, ss[gi], sc, ht_slots[gi])
                return new_cs

            def run_steps(nsteps, hs, cs, x_tiles, first_chunk=False):
                """nsteps of the recurrence with XB-step out-proj bursts."""
                for w in range(0, nsteps, XB):
                    nb = min(XB, nsteps - w)
                    hbufs = [hpool.tile([128, XB, BG], f32r, tag=f"hb{gi}",
                                        name=f"hb{gi}")
                             for gi in range(NG)]
                    for s_loc in range(nb):
                        slots = [hbufs[gi][:, s_loc, :] for gi in range(NG)]
                        cs = step(hs, cs, slots,
                                  first=(first_chunk and w == 0 and s_loc == 0),
                                  rot=(w * XB + s_loc) % NG)
                        hs = slots
                    for gi in range(NG):
                        x_burst(gi, hbufs[gi], x_tiles[gi], w, nb)
                return hs, cs

            def open_window(nsteps):
                """PSUM x tiles (per group, or per pair if XPAIR),
                bias-initialized via K=1 ones-matmuls."""
                x_tiles = []
                if XPAIR:
                    glen = NBT * nsteps * INPUT_SIZE
                    for p in range(NG // 2):
                        x_ps = ps_x.tile([128, 2 * glen], f32, name="x_ps")
                        nc.tensor.matmul(
                            x_ps, ones, brow[:, 0:2 * glen],
                            start=True, stop=False, skip_group_check=True,
                        )
                        x_tiles.append(x_ps)
                    return [x_tiles[gi // 2] for gi in range(NG)]
                for gi in range(NG):
                    x_ps = ps_x.tile([128, NBT * nsteps * INPUT_SIZE], f32)
                    nc.tensor.matmul(
                        x_ps, ones, brow[:, 0:NBT * nsteps * INPUT_SIZE],
                        start=True, stop=False, skip_group_check=True,
                    )
                    x_tiles.append(x_ps)
                return x_tiles

            def close_window(x_tiles, nsteps, ts, xsb_pool):
                if XPAIR:
                    for p in range(NG // 2):
                        x_sb = xsb_pool.tile(
                            [128, 2 * NBT * nsteps * INPUT_SIZE], f32,
                            tag=f"xsb{p}")
                        nc.vector.tensor_copy(x_sb, x_tiles[2 * p])
                        xv = x_sb.rearrange(
                            "p (h k s j) -> p (h k) s j", h=2, k=NBT,
                            j=INPUT_SIZE)
                        nc.sync.dma_start(
                            out=y_v[:, 2 * NBT * p:2 * NBT * (p + 1), ts, :],
                            in_=xv)
                    return
                for gi in range(NG):
                    x_sb = xsb_pool.tile(
                        [128, NBT * nsteps * INPUT_SIZE], f32, tag=f"xsb{gi}")
                    nc.vector.tensor_copy(x_sb, x_tiles[gi])
                    xv = x_sb.rearrange(
                        "p (k s j) -> p k s j", k=NBT, j=INPUT_SIZE)
                    nc.sync.dma_start(
                        out=y_v[:, NBT * gi:NBT * (gi + 1), ts, :], in_=xv)

            # ---- peeled chunk 0 ----
            cs = []
            for gi in range(NG):
                c0 = cpool.tile([128, BG], f32, name="c0")
                nc.vector.memset(c0, 0.0)
                cs.append(c0)
            hs0 = [hpool.tile([128, XB, BG], f32r, tag=f"hb{gi}", name=f"hb{gi}")
                   for gi in range(NG)]
            hs = [hs0[gi][:, XB - 1, :] for gi in range(NG)]
            x_tiles = open_window(u0)
            hs, cs = run_steps(u0, hs, cs, x_tiles, first_chunk=True)
            close_window(x_tiles, u0, slice(0, u0), xsb)

            # ---- chunks 1..n_chunks-1 ----
            if n_chunks > 1 and UNROLL:
                for ci in range(1, n_chunks):
                    x_tiles = open_window(U)
                    hs, cs = run_steps(U, hs, cs, x_tiles)
                    close_window(x_tiles, U, slice(ci * U, (ci + 1) * U), xsb)
            elif n_chunks > 1:
                with tc.For_i(1, n_chunks, 1, staggered_reset=STAGGER) as ci:
                    x_tiles = open_window(U)
                    hs2, cs2 = run_steps(U, hs, cs, x_tiles)
                    close_window(x_tiles, U, bass.ts(ci, U), xsb)
    nc.compile()
    return nc


_NC_CACHE = {}


def _get_nc():
    if "nc" not in _NC_CACHE:
        _NC_CACHE["nc"] = _build_nc()
    return _NC_CACHE["nc"]


def kernel(z, condition, start_point, W_ih, W_hh, b_ih, b_hh, W_out, b_out, seq_len):
    z = np.asarray(z, dtype=np.float32)
    condition = np.asarray(condition, dtype=np.float32)
    start_point = np.asarray(start_point, dtype=np.float32)
    W_ih = np.asarray(W_ih, dtype=np.float32)
    W_hh = np.asarray(W_hh, dtype=np.float32)
    b_ih = np.asarray(b_ih, dtype=np.float32)
    b_hh = np.asarray(b_hh, dtype=np.float32)
    W_out = np.asarray(W_out, dtype=np.float32)
    b_out = np.asarray(b_out, dtype=np.float32)
    assert int(seq_len) == T and z.shape[0] == B_FULL

    B = z.shape[0]
    dt_col = np.full((B, 1), 0.05, dtype=np.float32)
    static_in = np.concatenate([z, condition, dt_col], axis=-1)          # [B, 37]
    static_proj = static_in @ W_ih[:, INPUT_SIZE:].T + b_ih + b_hh       # [B, 512]
    Wx = W_ih[:, :INPUT_SIZE]                                            # [512, 2]
    P0 = static_proj + b_out @ Wx.T                                      # [B, 512]
    Pf = static_proj + start_point @ Wx.T                                # [B, 512]
    W_comb = (W_hh.T + W_out.T @ Wx.T).astype(np.float32)                # [128, 512]

    # h-state is ht = h/2  ->  double W_comb and W_out.T.
    # tanh(g) = 2 sig(2g) - 1  ->  double the g-gate block again (cols 256:384
    # of W_eff, rows 256:384 of P0/Pf).
    W_eff = 2.0 * W_comb
    W_eff[:, 2 * HIDDEN:3 * HIDDEN] *= 2.0
    P0e = P0.copy()
    P0e[:, 2 * HIDDEN:3 * HIDDEN] *= 2.0
    Pfe = Pf.copy()
    Pfe[:, 2 * HIDDEN:3 * HIDDEN] *= 2.0

    # unit-major, per-gate: [4, 128, B], gate order [i, f, g, o]
    P0_t = np.ascontiguousarray(P0e.T.reshape(4, HIDDEN, B), dtype=np.float32)
    Pf_t = np.ascontiguousarray(Pfe.T.reshape(4, HIDDEN, B), dtype=np.float32)
    # per-core per-group contiguous view for p0: [hid, NG, 4, BG]
    # P0_t[g, p, b] with b = core*BC + grp*BG + r  ->  p0[p, grp, g, r]
    P0_pg = np.ascontiguousarray(
        P0_t.reshape(4, HIDDEN, NCORES, NG, BG).transpose(2, 1, 3, 0, 4))
    woutT2 = np.ascontiguousarray(2.0 * W_out.T, dtype=np.float32)       # [128, 2]
    ident = np.eye(HIDDEN, dtype=np.float32)
    ones_row = np.ones((1, HIDDEN), dtype=np.float32)
    brow = np.tile(b_out, 256)[None, :].astype(np.float32)               # [1, 512]

    nc = _get_nc()
    in_maps = []
    for c in range(NCORES):
        bs = slice(c * BC, (c + 1) * BC)
        in_maps.append({
            "p0": P0_pg[c],
            "pf": np.ascontiguousarray(Pf_t[:, :, bs]),
            "weff": np.ascontiguousarray(W_eff),
            "woutT": woutT2,
            "ident": ident,
            "ones": ones_row,
            "brow": brow,
        })
    global _last_in_maps
    _last_in_maps = in_maps
    res = run_bass_kernel_spmd(nc, in_maps, core_ids=list(range(NCORES)))
    out = np.concatenate([r["y"] for r in res.results], axis=0)
    return out


_last_in_maps = None


# revision 39
# speedup vs baseline: 1.0904x; 1.0137x over previous
"""Trainium2 Bass kernel for nn_CVAE decoder LSTM.

Data-parallel over 8 NeuronCores: batch 8192 -> 1024 per core.

Math (per core, B=1024, T=2048, H=128). Reference step:
    gates = x_t @ Wx.T + static_proj + h @ W_hh.T
    i,f,g,o = split(gates); c' = sig(f)*c + sig(i)*tanh(g)
    h' = sig(o)*tanh(c'); x' = h' @ W_out.T + b_out; ys[t] = x'
Folds applied (all exact):
  1. x_t = h_t @ W_out.T + b_out for t>=1:
       W_comb = W_hh.T + W_out.T @ Wx.T        [128, 512]
       P0     = static_proj + b_out @ Wx.T     [B, 512]
       gates_t = h_t @ W_comb + P0   (t>=1);  gates_0 = Pf (h_0 = 0)
  2. tanh(x) = 2*sigmoid(2x) - 1: double the g-gate columns of W_comb/P0 so
     ONE sigmoid instruction covers all 4 gates; the affine 2s-1 rides the
     scalar_tensor_tensor multiplies on DVE.
  3. h = sig(o)*tanh(c) = 2*(sig(2c)-0.5)*sig(o) =: 2*ht. The x2 is folded
     into W_comb and W_out (both doubled), so the kernel state is
     ht = (sig(2c)-0.5)*sig(o), one STT op.
Per group (BG=256) per step:
  PE:   4 gate matmuls (W_eff) + 4 identity matmuls (P0 accumulate)
  ACT:  s = sigmoid(gates) over [128,4,BG] (ONE instr); sc = sigmoid(2c')
  Pool: t2 = s[f]*c
  DVE:  v = (s[g]-0.5)*s[i] ; c' = 2v + t2 ; ht = (sc-0.5)*s[o]
  PE:   x-projection into a per-group PSUM window tile [128,2,U,2]
        (bias pre-filled via a K=1 ones-matmul), emitted in XB=8-step
        BURSTS after the ht ring-buffer slots are all written -- a
        per-step out-proj would park an LDWEIGHTS waiting on ht at the
        PE queue head and stall the other groups' gate matmuls behind
        it (measured +2.5 ms). Window copied PSUM->SBUF, DMA'd per U.
Batch is split into NG=4 independent groups of BG=256 whose serial cell
chains stagger across engines. Chain: PE -> ACT -> {Pool,DVE} -> DVE ->
ACT -> DVE -> PE. ACT is the throughput bound (sim: 99.6% busy); NG=2
halves ACT instr overhead but the longer per-group chain latency loses
more (measured), and merging sigmoid_c across groups couples chains
(also measured worse). Failed variants (measured slower): c-add on Pool
(17.3ms), XB=4 (15.9ms), no-P0-ident (17.9ms), sc pair-merge (sim 15.1),
XPAIR pair-shared x windows + 3 gate bufs at U=64 (20.3ms -- bank-overlap
serialization of the paired groups' PSUM writes).
Emission order matters: interleaving each group's sigmoid_c/ht emission
immediately after its phase1 (ILV=1, default) measured 13.7ms vs 14.7ms
for phase-grouped emission -- same instructions and deps, different
TileScheduler priority order. Rotating the group emission order per step
(ROT=1) breaks the stagger and measures 17.3ms; XB=10 ht rings overflow
SBUF. The fixed order 0..3 with ILV is the tuned schedule.
"""

import os
import numpy as np

import concourse.bass as bass
import concourse.bacc as bacc
import concourse.tile as tile
from concourse import mybir
from concourse.bass_utils import run_bass_kernel_spmd

f32 = mybir.dt.float32
f32r = mybir.dt.float32r
AF = mybir.ActivationFunctionType
ALU = mybir.AluOpType

HIDDEN = 128
INPUT_SIZE = 2
T = 2048
B_FULL = 8192
NCORES = 8
BC = B_FULL // NCORES      # 1024 batch rows per core
NG = 4                     # independent batch groups per core
BG = BC // NG              # 256 rows per group
NBT = BG // 128            # 2 batch-tiles of 128 per group
U = int(os.environ.get("K_U", "128"))  # steps per loop chunk / x window
STAGGER = os.environ.get("K_STAGGER", "1") == "1"
UNROLL = os.environ.get("K_UNROLL", "") == "1"  # python-unroll For_i (for sim)
GATE_BUFS = int(os.environ.get("K_GATE_BUFS", "0")) or 0
NO_P0 = os.environ.get("K_NO_P0", "") == "1"   # diagnostic: skip ident matmuls
NO_X = os.environ.get("K_NO_X", "") == "1"     # diagnostic: skip x projection
XB = int(os.environ.get("K_XB", "8"))          # out-projection burst size (steps)
CADD_POOL = os.environ.get("K_CADD_POOL", "") == "1"  # c'=2v+t2 on Pool vs DVE
# pair-shared x windows (1 PSUM bank per 2 groups) freeing banks for a 3rd
# gates buffer; requires U=64 so a pair window fits one bank
XPAIR = os.environ.get("K_XPAIR", "") == "1"
ILV = os.environ.get("K_ILV", "1") == "1"      # interleave sc/phase2 per group
HB = int(os.environ.get("K_HB", "2"))          # ht ring buffers
ROT = os.environ.get("K_ROT", "") == "1"       # rotate group emission order
BDELAY = os.environ.get("K_BDELAY", "") == "1"  # emit bursts 1 step delayed


def _build_nc(t_total=T):
    nc = bacc.Bacc("TRN2", target_bir_lowering=False)

    # per-group contiguous: [hid, group, gate, batch-in-group]
    p0_d = nc.dram_tensor("p0", [HIDDEN, NG, 4, BG], f32r, kind="ExternalInput")
    pf_d = nc.dram_tensor("pf", [4, HIDDEN, BC], f32, kind="ExternalInput")
    weff_d = nc.dram_tensor("weff", [HIDDEN, 4 * HIDDEN], f32r, kind="ExternalInput")
    woutT_d = nc.dram_tensor("woutT", [HIDDEN, INPUT_SIZE], f32r, kind="ExternalInput")
    ident_d = nc.dram_tensor("ident", [HIDDEN, HIDDEN], f32r, kind="ExternalInput")
    ones_d = nc.dram_tensor("ones", [1, HIDDEN], f32r, kind="ExternalInput")
    brow_d = nc.dram_tensor("brow", [1, 512], f32r, kind="ExternalInput")
    y_d = nc.dram_tensor("y", [BC, t_total, INPUT_SIZE], f32, kind="ExternalOutput")
    # [p, k, t, j] view of y: batch row b = k*128 + p
    y_v = y_d.ap().rearrange("(k p) t j -> p k t j", p=128)

    gate_bufs = GATE_BUFS or (3 if XPAIR else 2)
    u0 = min(U, t_total)
    n_chunks = t_total // U if t_total >= U else 1

    with tile.TileContext(nc) as tc:
        with (
            tc.tile_pool(name="consts", bufs=1) as consts,
            tc.tile_pool(name="hpool", bufs=HB) as hpool,
            tc.tile_pool(name="cpool", bufs=2 * NG) as cpool,
            tc.tile_pool(name="cell", bufs=2 * NG) as cell,
            tc.tile_pool(name="ps_gate", bufs=gate_bufs, space="PSUM") as ps_gate,
            tc.tile_pool(name="ps_x", bufs=(NG // 2 if XPAIR else NG),
                         space="PSUM") as ps_x,
            tc.tile_pool(name="xsb", bufs=2) as xsb,
        ):
            # ---- constants ----
            weff = consts.tile([HIDDEN, 4 * HIDDEN], f32r)
            woutT = consts.tile([HIDDEN, INPUT_SIZE], f32r)
            ident = consts.tile([HIDDEN, HIDDEN], f32r)
            ones = consts.tile([1, HIDDEN], f32r)
            brow = consts.tile([1, 512], f32r)
            p0 = consts.tile([128, NG, 4, BG], f32r)  # [hid, group, gate, b-in-g]
            nc.gpsimd.dma_start(out=weff, in_=weff_d[:, :])
            nc.gpsimd.dma_start(out=woutT, in_=woutT_d[:, :])
            nc.gpsimd.dma_start(out=ident, in_=ident_d[:, :])
            nc.gpsimd.dma_start(out=ones, in_=ones_d[:, :])
            nc.gpsimd.dma_start(out=brow, in_=brow_d[:, :])
            nc.gpsimd.dma_start(out=p0, in_=p0_d.ap())

            def group_phase1(gi, h_prev, c_prev, c_new, first):
                """Gates + sigmoid + c-update for group gi. Returns s."""
                bs = slice(gi * BG, (gi + 1) * BG)
                if first:
                    gates = cell.tile([128, 4, BG], f32, tag="gates_f")
                    nc.sync.dma_start(
                        out=gates,
                        in_=pf_d.ap().rearrange("g p b -> p g b")[:, :, bs],
                    )
                else:
                    gates = ps_gate.tile([128, 4, BG], f32)
                    for g in range(4):
                        nc.tensor.matmul(
                            gates[:, g, :],
                            weff[:, g * 128:(g + 1) * 128],
                            h_prev[:, :],
                            start=True, stop=NO_P0,
                        )
                        if not NO_P0:
                            nc.tensor.matmul(
                                gates[:, g, :], ident, p0[:, gi, g, :],
                                start=False, stop=True,
                            )

                s = cell.tile([128, 4, BG], f32, tag="s")
                nc.scalar.activation(s, gates, AF.Sigmoid)

                # t2 = s[f]*c on Pool ; v = (s[g]-0.5)*s[i] on DVE
                t2 = cell.tile([128, BG], f32, tag="t2")
                nc.gpsimd.tensor_mul(t2, s[:, 1, :], c_prev)
                v = cell.tile([128, BG], f32, tag="v")
                nc.vector.scalar_tensor_tensor(
                    v, s[:, 2, :], 0.5, s[:, 0, :],
                    op0=ALU.subtract, op1=ALU.mult)
                if CADD_POOL:
                    # c-state is c/2: ct' = v + sf*ct ; sigma_c uses scale=4
                    nc.gpsimd.tensor_add(c_new, v, t2)
                else:
                    nc.vector.scalar_tensor_tensor(
                        c_new, v, 2.0, t2, op0=ALU.mult, op1=ALU.add)
                return s

            def group_phase2(gi, s, sc, ht_slot):
                """ht = (sc-0.5)*s[o] written into the group's ring buffer."""
                nc.vector.scalar_tensor_tensor(
                    ht_slot, sc, 0.5, s[:, 3, :], op0=ALU.subtract, op1=ALU.mult)

            def x_burst(gi, hbuf, x_ps, s_base, nsteps):
                """Out-projections for steps [s_base, s_base+nsteps) of group gi,
                emitted after all their ht slots are written (no PE queue
                stalls on ht waits)."""
                if NO_X:
                    return
                if XPAIR:
                    xv = x_ps.rearrange(
                        "p (h k s j) -> p h k s j", h=2, k=NBT,
                        j=INPUT_SIZE)[:, gi % 2]
                else:
                    xv = x_ps.rearrange(
                        "p (k s j) -> p k s j", k=NBT, j=INPUT_SIZE)
                for s_loc in range(nsteps):
                    for k in range(NBT):
                        nc.tensor.matmul(
                            xv[:, k, s_base + s_loc, :],
                            hbuf[:, s_loc, k * 128:(k + 1) * 128],
                            woutT,
                            start=False, stop=True, skip_group_check=True,
                        )

            def step(hs, cs, ht_slots, first=False, rot=0):
                """One LSTM step; ht written into ht_slots[gi]. Returns cs."""
                new_cs = [cpool.tile([128, BG], f32, name="cnew")
                          for _ in range(NG)]
                order = [(g + rot) % NG for g in range(NG)] if ROT else range(NG)
                if ILV:
                    for gi in order:
                        s = group_phase1(gi, hs[gi], cs[gi], new_cs[gi], first)
                        sc = cell.tile([128, BG], f32, tag="sc")
                        nc.scalar.activation(sc, new_cs[gi], AF.Sigmoid,
                                             scale=4.0 if CADD_POOL else 2.0)
                        group_phase2(gi, s, sc, ht_slots[gi])
                    return new_cs
                ss = [group_phase1(gi, hs[gi], cs[gi], new_cs[gi], first)
                      for gi in range(NG)]
                for gi in range(NG):
                    sc = cell.tile([128, BG], f32, tag="sc")
                    nc.scalar.activation(sc, new_cs[gi], AF.Sigmoid,
                                         scale=4.0 if CADD_POOL else 2.0)
                    group_phase2(gi, ss[gi], sc, ht_slots[gi])
                return new_cs

            def run_steps(nsteps, hs, cs, x_tiles, first_chunk=False):
                """nsteps of the recurrence with XB-step out-proj bursts.
                With BDELAY the bursts are emitted one step into the next
                window so the scheduler slots them into PE gaps there."""
                pending = None
                for w in range(0, nsteps, XB):
                    nb = min(XB, nsteps - w)
                    hbufs = [hpool.tile([128, XB, BG], f32r, tag=f"hb{gi}",
                                        name=f"hb{gi}")
                             for gi in range(NG)]
                    for s_loc in range(nb):
                        slots = [hbufs[gi][:, s_loc, :] for gi in range(NG)]
                        cs = step(hs, cs, slots,
                                  first=(first_chunk and w == 0 and s_loc == 0),
                                  rot=(w * XB + s_loc) % NG)
                        hs = slots
                        if s_loc == 0 and pending is not None:
                            pw, pb, pbufs = pending
                            for gi in range(NG):
                                x_burst(gi, pbufs[gi], x_tiles[gi], pw, pb)
                            pending = None
                    if BDELAY:
                        pending = (w, nb, hbufs)
                    else:
                        for gi in range(NG):
                            x_burst(gi, hbufs[gi], x_tiles[gi], w, nb)
                if pending is not None:
                    pw, pb, pbufs = pending
                    for gi in range(NG):
                        x_burst(gi, pbufs[gi], x_tiles[gi], pw, pb)
                return hs, cs

            def open_window(nsteps):
                """PSUM x tiles (per group, or per pair if XPAIR),
                bias-initialized via K=1 ones-matmuls."""
                x_tiles = []
                if XPAIR:
                    glen = NBT * nsteps * INPUT_SIZE
                    for p in range(NG // 2):
                        x_ps = ps_x.tile([128, 2 * glen], f32, name="x_ps")
                        nc.tensor.matmul(
                            x_ps, ones, brow[:, 0:2 * glen],
                            start=True, stop=False, skip_group_check=True,
                        )
                        x_tiles.append(x_ps)
                    return [x_tiles[gi // 2] for gi in range(NG)]
                for gi in range(NG):
                    x_ps = ps_x.tile([128, NBT * nsteps * INPUT_SIZE], f32)
                    nc.tensor.matmul(
                        x_ps, ones, brow[:, 0:NBT * nsteps * INPUT_SIZE],
                        start=True, stop=False, skip_group_check=True,
                    )
                    x_tiles.append(x_ps)
                return x_tiles

            def close_window(x_tiles, nsteps, ts, xsb_pool):
                if XPAIR:
                    for p in range(NG // 2):
                        x_sb = xsb_pool.tile(
                            [128, 2 * NBT * nsteps * INPUT_SIZE], f32,
                            tag=f"xsb{p}")
                        nc.vector.tensor_copy(x_sb, x_tiles[2 * p])
                        xv = x_sb.rearrange(
                            "p (h k s j) -> p (h k) s j", h=2, k=NBT,
                            j=INPUT_SIZE)
                        nc.sync.dma_start(
                            out=y_v[:, 2 * NBT * p:2 * NBT * (p + 1), ts, :],
                            in_=xv)
                    return
                for gi in range(NG):
                    x_sb = xsb_pool.tile(
                        [128, NBT * nsteps * INPUT_SIZE], f32, tag=f"xsb{gi}")
                    nc.vector.tensor_copy(x_sb, x_tiles[gi])
                    xv = x_sb.rearrange(
                        "p (k s j) -> p k s j", k=NBT, j=INPUT_SIZE)
                    nc.sync.dma_start(
                        out=y_v[:, NBT * gi:NBT * (gi + 1), ts, :], in_=xv)

            # ---- peeled chunk 0 ----
            cs = []
            for gi in range(NG):
                c0 = cpool.tile([128, BG], f32, name="c0")
                nc.vector.memset(c0, 0.0)
                cs.append(c0)
            hs0 = [hpool.tile([128, XB, BG], f32r, tag=f"hb{gi}", name=f"hb{gi}")
                   for gi in range(NG)]
            hs = [hs0[gi][:, XB - 1, :] for gi in range(NG)]
            x_tiles = open_window(u0)
            hs, cs = run_steps(u0, hs, cs, x_tiles, first_chunk=True)
            close_window(x_tiles, u0, slice(0, u0), xsb)

            # ---- chunks 1..n_chunks-1 ----
            if n_chunks > 1 and UNROLL:
                for ci in range(1, n_chunks):
                    x_tiles = open_window(U)
                    hs, cs = run_steps(U, hs, cs, x_tiles)
                    close_window(x_tiles, U, slice(ci * U, (ci + 1) * U), xsb)
            elif n_chunks > 1:
                with tc.For_i(1, n_chunks, 1, staggered_reset=STAGGER) as ci:
                    x_tiles = open_window(U)
                    hs2, cs2 = run_steps(U, hs, cs, x_tiles)
                    close_window(x_tiles, U, bass.ts(ci, U), xsb)
    nc.compile()
    return nc


_NC_CACHE = {}


def _get_nc():
    if "nc" not in _NC_CACHE:
        _NC_CACHE["nc"] = _build_nc()
    return _NC_CACHE["nc"]


def kernel(z, condition, start_point, W_ih, W_hh, b_ih, b_hh, W_out, b_out, seq_len):
    z = np.asarray(z, dtype=np.float32)
    condition = np.asarray(condition, dtype=np.float32)
    start_point = np.asarray(start_point, dtype=np.float32)
    W_ih = np.asarray(W_ih, dtype=np.float32)
    W_hh = np.asarray(W_hh, dtype=np.float32)
    b_ih = np.asarray(b_ih, dtype=np.float32)
    b_hh = np.asarray(b_hh, dtype=np.float32)
    W_out = np.asarray(W_out, dtype=np.float32)
    b_out = np.asarray(b_out, dtype=np.float32)
    assert int(seq_len) == T and z.shape[0] == B_FULL

    B = z.shape[0]
    dt_col = np.full((B, 1), 0.05, dtype=np.float32)
    static_in = np.concatenate([z, condition, dt_col], axis=-1)          # [B, 37]
    static_proj = static_in @ W_ih[:, INPUT_SIZE:].T + b_ih + b_hh       # [B, 512]
    Wx = W_ih[:, :INPUT_SIZE]                                            # [512, 2]
    P0 = static_proj + b_out @ Wx.T                                      # [B, 512]
    Pf = static_proj + start_point @ Wx.T                                # [B, 512]
    W_comb = (W_hh.T + W_out.T @ Wx.T).astype(np.float32)                # [128, 512]

    # h-state is ht = h/2  ->  double W_comb and W_out.T.
    # tanh(g) = 2 sig(2g) - 1  ->  double the g-gate block again (cols 256:384
    # of W_eff, rows 256:384 of P0/Pf).
    W_eff = 2.0 * W_comb
    W_eff[:, 2 * HIDDEN:3 * HIDDEN] *= 2.0
    P0e = P0.copy()
    P0e[:, 2 * HIDDEN:3 * HIDDEN] *= 2.0
    Pfe = Pf.copy()
    Pfe[:, 2 * HIDDEN:3 * HIDDEN] *= 2.0

    # unit-major, per-gate: [4, 128, B], gate order [i, f, g, o]
    P0_t = np.ascontiguousarray(P0e.T.reshape(4, HIDDEN, B), dtype=np.float32)
    Pf_t = np.ascontiguousarray(Pfe.T.reshape(4, HIDDEN, B), dtype=np.float32)
    # per-core per-group contiguous view for p0: [hid, NG, 4, BG]
    # P0_t[g, p, b] with b = core*BC + grp*BG + r  ->  p0[p, grp, g, r]
    P0_pg = np.ascontiguousarray(
        P0_t.reshape(4, HIDDEN, NCORES, NG, BG).transpose(2, 1, 3, 0, 4))
    woutT2 = np.ascontiguousarray(2.0 * W_out.T, dtype=np.float32)       # [128, 2]
    ident = np.eye(HIDDEN, dtype=np.float32)
    ones_row = np.ones((1, HIDDEN), dtype=np.float32)
    brow = np.tile(b_out, 256)[None, :].astype(np.float32)               # [1, 512]

    nc = _get_nc()
    in_maps = []
    for c in range(NCORES):
        bs = slice(c * BC, (c + 1) * BC)
        in_maps.append({
            "p0": P0_pg[c],
            "pf": np.ascontiguousarray(Pf_t[:, :, bs]),
            "weff": np.ascontiguousarray(W_eff),
            "woutT": woutT2,
            "ident": ident,
            "ones": ones_row,
            "brow": brow,
        })
    global _last_in_maps
    _last_in_maps = in_maps
    res = run_bass_kernel_spmd(nc, in_maps, core_ids=list(range(NCORES)))
    out = np.concatenate([r["y"] for r in res.results], axis=0)
    return out


_last_in_maps = None
